# revision 24
# baseline (speedup 1.0000x reference)
"""Mixture-of-Softmaxes Trainium2 kernel (Bass/Tile, 8-core data parallel).

Reference computation (per token t, hidden h[1024]):
  prior  = sigmoid(h @ prior_w + prior_b); prior /= (prior.sum(heads) + 1e-8)
  latent = tanh(h @ latent_w + latent_b).reshape(8, 1024)
  logits = latent @ output_w + output_b                # [8, 2048]
  out    = sum_n prior[n] * softmax(logits[n])         # [2048]

Sharding: data-parallel over the 8192 tokens (B*S), 1024 tokens/core.
All params replicated. Matmul inputs fp16, fp32 PSUM accumulation;
fp16 accumulator/output (host upcasts to fp32).

The kernel is PE-bound (~95% TensorMatrix occupancy): 3072 back-to-back
[128x128]x[128x512] fp16 matmuls. The remaining overhead is the startup
window (fixed ~9us DMA bring-up + first-tile transfer) and the end-of-
kernel drain, so the input DMAs are priority-ordered by first-use time:
  - prior_w (2 KB) first on the sync queue: the first PE work (prior
    matmuls) needs only it + hidden supertile 0.
  - hidden supertile 0 in two kc-halves concurrently on sync + gpsimd.
  - latent_w in quarter tiles: phase A's first matmul group waits on a
    single 0.5 MB quarter, not the full 2.1 MB head.
  - output_w as per-hd-chunk tiles alternating across sync/gpsimd right
    behind the hidden halves; phase-B matmuls wait on exactly the chunk
    they consume.
  - hidden supertile 1 (needed at mid-kernel) queued last.

Device layout (per core, T=1024 tokens):
  hiddenT [P, KC, T-tile] (host pre-transposed) so both big matmuls need
  no on-device transpose:
    phase A: latT[hd] = (latent_w[:, n*H+hd*128 : +128]).T @ hiddenT -> [128, T]
             tanh -> fp16
    phase B: logits = latT_tile.T @ output_w -> [128 tok, V] in PSUM
  softmax w/o max-subtract (logits ~ N(0, 0.63^2), exp is safe), denom via
  the ACT accum_out side-output; per-head combine is one fused DVE
  scalar_tensor_tensor: acc = (E * w_n) + acc.  The last head's combine +
  output DMA run in 512-column chunks to shorten the end-of-kernel drain.
"""

import os
import numpy as np
import ml_dtypes

B, S, H, NH, V = 4, 2048, 1024, 8, 2048
N_CORES = 8
T = (B * S) // N_CORES          # tokens per core
P = 128
KH = H // P                     # 8 contraction chunks
ST = 512                        # phase-A moving (token) tile
N_ST = T // ST
TT_PER_ST = ST // P
N_TT = T // P
VC = 512                        # logits free-dim chunk (one PSUM bank)
NVC = V // VC
NLQ = 4                         # latent_w quarter tiles per head
LQW = H // NLQ                  # 256 columns per quarter
EPS = 1e-8

_CACHE = {}


def _build(with_bias):
    import concourse.bass as bass
    import concourse.mybir as mybir
    import concourse.tile as tile
    from concourse import bacc
    from concourse.bass import ts

    f32 = mybir.dt.float32
    bf16 = mybir.dt.float16  # fp16: same PE rate as bf16, 8x finer mantissa

    KC = KH + (1 if with_bias else 0)   # contraction chunks incl. bias row
    HD = KH + (1 if with_bias else 0)   # logits contraction chunks
    HPQ = KH // NLQ                     # hd slices per latent_w quarter
    # hidden kc-half split: both consumers (prior lhsT, phase-A rhs)
    # address hidden per-kc, so separate half tiles give precise
    # DMA->matmul dependencies at zero extra matmul cost.
    NHQ = 2
    KQ = [KC - KC // 2, KC // 2]
    KOFF = [sum(KQ[:i]) for i in range(NHQ)]
    KIDX = []                           # kc -> (half, local index)
    for qi, k in enumerate(KQ):
        for j in range(k):
            KIDX.append((qi, j))

    nc = bacc.Bacc("TRN2", target_bir_lowering=False, debug=False)

    hTq_d = [
        nc.dram_tensor(f"hiddenT{qi}", [N_ST, P, KQ[qi], ST], bf16,
                       kind="ExternalInput")
        for qi in range(NHQ)
    ]
    pw_d = nc.dram_tensor("prior_w", [P, KC, NH], bf16, kind="ExternalInput")
    lw_d = nc.dram_tensor("latent_w", [NH, NLQ, P, KC, LQW], bf16,
                          kind="ExternalInput")
    ow_d = nc.dram_tensor("output_w", [HD, P, V], bf16, kind="ExternalInput")
    out_d = nc.dram_tensor("out", [T, V], bf16, kind="ExternalOutput")

    with tile.TileContext(nc) as tc:
        with (
            tc.tile_pool(name="const", bufs=1) as const,
            tc.tile_pool(name="hid", bufs=N_ST) as hpool,
            tc.tile_pool(name="oww", bufs=1) as owpool,
            tc.tile_pool(name="pww", bufs=1) as pwpool,
            tc.tile_pool(name="lww", bufs=2) as lwpool,
            tc.tile_pool(name="lat", bufs=2 * KH) as latpool,
            tc.tile_pool(name="ee", bufs=3) as epool,
            tc.tile_pool(name="acc", bufs=TT_PER_ST + 1) as accpool,
            tc.tile_pool(name="small", bufs=4 * N_TT) as spool,
            tc.tile_pool(name="ps_lat", bufs=4, space="PSUM") as ps_lat,
            tc.tile_pool(name="ps_log", bufs=2, space="PSUM") as ps_log,
            tc.tile_pool(name="ps_pri", bufs=2, space="PSUM") as ps_pri,
        ):
            # ---- resident loads: priority order = first-use order --------
            pw = pwpool.tile([P, KC, NH], bf16, tag="pw")
            nc.sync.dma_start(pw[:], pw_d[:])

            hTs = []
            for sti in range(N_ST):
                hTs.append([hpool.tile([P, KQ[qi], ST], bf16,
                                       name=f"hT{qi}", tag=f"hT{qi}")
                            for qi in range(NHQ)])

            def h_ap(sti, kc, cols=None):
                qi, j = KIDX[kc]
                t = hTs[sti][qi]
                return t[:, j, :] if cols is None else t[:, j, cols]

            # supertile-0 halves concurrently on the sync and gpsimd
            # queues; the prior / phase-A kc chains consume them as they
            # land.
            nc.sync.dma_start(hTs[0][0][:], hTq_d[0][0])
            nc.gpsimd.dma_start(hTs[0][1][:], hTq_d[1][0])

            def load_lw(n, engine=None):
                tiles = []
                for qi in range(NLQ):
                    t = lwpool.tile([P, KC, LQW], bf16, tag=f"lw{qi}")
                    (engine or nc.sync).dma_start(t[:], lw_d[n, qi])
                    tiles.append(t)
                return tiles

            lw_next = load_lw(0, engine=nc.scalar)

            # output_w per-hd chunks, alternating sync/gpsimd right behind
            # the hidden halves; consumed in hd order by phase B.
            ow_t = []
            for hd in range(HD):
                t = owpool.tile([P, V], bf16, tag=f"ow{hd}")
                (nc.sync if hd % 2 == 0 else nc.gpsimd).dma_start(
                    t[:], ow_d[hd])
                ow_t.append(t)

            # supertile-1 hidden: needed at mid-kernel, queued last.
            for qi in range(NHQ):
                nc.gpsimd.dma_start(hTs[1][qi][:], hTq_d[qi][1])

            if with_bias:
                ones_t = const.tile([P, P], bf16)
                nc.vector.memset(ones_t[:], 0.0)
                nc.vector.memset(ones_t[0:1, :], 1.0)

            # ---- prior: wgt[tt] = sigmoid(h@pw) / (sum + EPS) -------------
            # emitted per supertile so the PE never waits on hT[st1]
            wgt = [None] * N_TT

            def prior_for(sti):
              for tt in range(sti * TT_PER_ST, (sti + 1) * TT_PER_ST):
                pr_ps = ps_pri.tile([P, NH], f32, tag="pri")
                for kc in range(KC):
                    nc.tensor.matmul(
                        pr_ps[:],
                        h_ap(tt // TT_PER_ST, kc, ts(tt % TT_PER_ST, P)),
                        pw[:, kc, :],
                        start=(kc == 0),
                        stop=(kc == KC - 1),
                    )
                sig = spool.tile([P, NH], f32, tag="sig")
                ssum = spool.tile([P, 1], f32, tag="ssum")
                nc.scalar.activation(
                    sig[:], pr_ps[:], mybir.ActivationFunctionType.Sigmoid,
                    accum_out=ssum[:],
                )
                nc.vector.tensor_scalar_add(ssum[:], ssum[:], float(EPS))
                inv = spool.tile([P, 1], f32, tag="inv")
                nc.vector.reciprocal(inv[:], ssum[:])
                w = spool.tile([P, NH], f32, tag="wgt")
                nc.vector.tensor_scalar_mul(w[:], sig[:], inv[:])
                wgt[tt] = w

            # ---- main: per 512-token supertile, per head ------------------
            for st in range(N_ST):
                prior_for(st)
                acc = {}
                for n in range(NH):
                    # phase A: latT[hd] [128, ST] fp16 = tanh(lw_n.T @ hT_st)
                    lw_n = lw_next
                    if not (st == N_ST - 1 and n == NH - 1):
                        lw_next = load_lw((n + 1) % NH)
                    latT = []
                    for hd in range(KH):
                        lw_q = lw_n[hd // HPQ]
                        hcol = hd % HPQ
                        lat_ps = ps_lat.tile([P, ST], f32, tag="lat")
                        for kc in range(KC):
                            nc.tensor.matmul(
                                lat_ps[:],
                                lw_q[:, kc, ts(hcol, P)],
                                h_ap(st, kc),
                                start=(kc == 0),
                                stop=(kc == KC - 1),
                            )
                        lt = latpool.tile([P, ST], bf16, tag="latT")
                        nc.scalar.activation(
                            lt[:], lat_ps[:], mybir.ActivationFunctionType.Tanh
                        )
                        latT.append(lt)

                    # phase B: per V-quarter [P,512] PSUM (1 bank,
                    # double-buffered): exp(q) overlaps the matmuls of q+1.
                    # E collects the full row; denom = reduce over the four
                    # per-quarter accum_outs.
                    for tti in range(TT_PER_ST):
                        tt = st * TT_PER_ST + tti
                        E = epool.tile([P, V], bf16, tag="E")
                        ds4 = spool.tile([P, NVC], f32, tag="ds4")
                        for q in range(NVC):
                            lg_ps = ps_log.tile([P, VC], f32, tag="log")
                            for hd in range(HD):
                                lhsT = (
                                    latT[hd][:, ts(tti, P)]
                                    if hd < KH
                                    else ones_t[:]
                                )
                                nc.tensor.matmul(
                                    lg_ps[:],
                                    lhsT,
                                    ow_t[hd][:, ts(q, VC)],
                                    start=(hd == 0),
                                    stop=(hd == HD - 1),
                                )
                            nc.scalar.activation(
                                E[:, ts(q, VC)], lg_ps[:],
                                mybir.ActivationFunctionType.Exp,
                                accum_out=ds4[:, q:q + 1],
                            )
                        dsm = spool.tile([P, 1], f32, tag="dsm")
                        nc.vector.tensor_reduce(
                            dsm[:], ds4[:], axis=mybir.AxisListType.X,
                            op=mybir.AluOpType.add,
                        )
                        invd = spool.tile([P, 1], f32, tag="invd")
                        nc.vector.reciprocal(invd[:], dsm[:])
                        wn = spool.tile([P, 1], f32, tag="wn")
                        nc.vector.tensor_tensor(
                            wn[:], wgt[tt][:, n:n + 1], invd[:],
                            op=mybir.AluOpType.mult,
                        )
                        if n == 0:
                            a = accpool.tile([P, V], bf16, tag="acc")
                            acc[tti] = a
                        else:
                            a = acc[tti]
                        # last head: 512-wide chunks so each chunk's output
                        # DMA starts as soon as its combine is done
                        # (shorter end-of-kernel drain); other heads: 1024.
                        nch = NVC if n == NH - 1 else 2
                        cw = V // nch
                        for ci in range(nch):
                            tgt = a[:, ts(ci, cw)]
                            Eh = E[:, ts(ci, cw)]
                            if n == 0:
                                nc.vector.tensor_scalar_mul(tgt, Eh, wn[:])
                            else:
                                nc.vector.scalar_tensor_tensor(
                                    tgt, Eh, wn[:], tgt,
                                    op0=mybir.AluOpType.mult,
                                    op1=mybir.AluOpType.add,
                                )
                            if n == NH - 1:
                                nc.sync.dma_start(
                                    out_d[ts(tt, P), ts(ci, cw)], tgt
                                )

    nc.compile()
    return nc


def _prep_inputs(hidden, prior_w, prior_b, latent_w, latent_b, output_w,
                 output_b, with_bias):
    """Rearrange inputs into the device (partition-major) layouts:
      hiddenTa/b [N_ST, P, KA/KB, ST] per core (kc-halves, contiguous)
      prior_w    [P, KC, NH]
      latent_w   [NH, NLQ, P, KC, LQW]  (H-quarters, contiguous)
      output_w   [HD, P, V]             (per-hd chunks, contiguous)
    """
    bf16 = np.float16
    KC = KH + (1 if with_bias else 0)
    HD = KH + (1 if with_bias else 0)
    KQ = [KC - KC // 2, KC // 2]
    KOFF = [0, KQ[0]]
    BS = B * S

    h = hidden.reshape(BS, H).astype(bf16)
    pw = prior_w.astype(bf16)
    lw = latent_w.astype(bf16)
    ow = output_w.astype(bf16)
    if with_bias:
        hx = np.zeros((BS, P), bf16)
        hx[:, 0] = 1.0
        h = np.concatenate([h, hx], axis=1)                   # [BS, KC*P]
        pw = np.concatenate(
            [pw, prior_b.astype(bf16)[None, :], np.zeros((P - 1, NH), bf16)],
            axis=0)
        lw = np.concatenate(
            [lw, latent_b.astype(bf16)[None, :],
             np.zeros((P - 1, NH * H), bf16)], axis=0)
        ow = np.concatenate(
            [ow, output_b.astype(bf16)[None, :], np.zeros((P - 1, V), bf16)],
            axis=0)

    hT = h.reshape(BS, KC, P).transpose(2, 1, 0)              # [P, KC, BS]
    pw_dev = np.ascontiguousarray(pw.reshape(KC, P, NH).transpose(1, 0, 2))
    lw_dev = np.ascontiguousarray(
        lw.reshape(KC, P, NH, NLQ, LQW).transpose(2, 3, 1, 0, 4))
    ow_dev = np.ascontiguousarray(ow.reshape(HD, P, V))
    return hT, pw_dev, lw_dev, ow_dev, KQ, KOFF


def kernel(hidden, prior_w, prior_b, latent_w, latent_b, output_w, output_b,
           _profile=False):
    from concourse.bass_utils import run_bass_kernel_spmd

    # coerce to host numpy (the caller may hand us jax arrays)
    hidden = np.asarray(hidden, dtype=np.float32)
    prior_w = np.asarray(prior_w, dtype=np.float32)
    prior_b = np.asarray(prior_b, dtype=np.float32)
    latent_w = np.asarray(latent_w, dtype=np.float32)
    latent_b = np.asarray(latent_b, dtype=np.float32)
    output_w = np.asarray(output_w, dtype=np.float32)
    output_b = np.asarray(output_b, dtype=np.float32)

    with_bias = bool(
        np.any(prior_b) or np.any(latent_b) or np.any(output_b)
    )
    key = with_bias
    if key not in _CACHE:
        _CACHE[key] = _build(with_bias)
    nc = _CACHE[key]

    hT, pw, lw, ow, KQ, KOFF = _prep_inputs(
        hidden, prior_w, prior_b, latent_w, latent_b, output_w, output_b,
        with_bias)

    in_maps = []
    for c in range(N_CORES):
        hc = hT[:, :, c * T:(c + 1) * T]                      # [P, KC, T]
        hc = np.ascontiguousarray(
            hc.reshape(P, hc.shape[1], N_ST, ST).transpose(2, 0, 1, 3))
        m = {
            "prior_w": pw,
            "latent_w": lw,
            "output_w": ow,
        }
        for qi in range(len(KQ)):
            m[f"hiddenT{qi}"] = np.ascontiguousarray(
                hc[:, :, KOFF[qi]:KOFF[qi] + KQ[qi], :])
        in_maps.append(m)

    res = run_bass_kernel_spmd(
        nc, in_maps, list(range(N_CORES)), trace=_profile
    )
    out = np.concatenate([res.results[c]["out"] for c in range(N_CORES)],
                         axis=0).astype(np.float32)
    if _profile:
        kernel.last_result = res
    return out.reshape(B, S, V)


# revision 25
# speedup vs baseline: 1.0023x; 1.0023x over previous
"""Mixture-of-Softmaxes Trainium2 kernel (Bass/Tile, 8-core data parallel).

Reference computation (per token t, hidden h[1024]):
  prior  = sigmoid(h @ prior_w + prior_b); prior /= (prior.sum(heads) + 1e-8)
  latent = tanh(h @ latent_w + latent_b).reshape(8, 1024)
  logits = latent @ output_w + output_b                # [8, 2048]
  out    = sum_n prior[n] * softmax(logits[n])         # [2048]

Sharding: data-parallel over the 8192 tokens (B*S), 1024 tokens/core.
All params replicated. Matmul inputs fp16, fp32 PSUM accumulation;
fp16 accumulator/output (host upcasts to fp32).

The kernel is PE-bound (~95% TensorMatrix occupancy): 3072 back-to-back
[128x128]x[128x512] fp16 matmuls. The remaining overhead is the startup
window (fixed ~9us DMA bring-up + first-tile transfer) and the end-of-
kernel drain, so the input DMAs are priority-ordered by first-use time:
  - prior_w (2 KB) first on the sync queue: the first PE work (prior
    matmuls) needs only it + hidden supertile 0.
  - hidden supertile 0 in two kc-halves concurrently on sync + gpsimd.
  - latent_w in quarter tiles: phase A's first matmul group waits on a
    single 0.5 MB quarter, not the full 2.1 MB head.
  - output_w as per-hd-chunk tiles alternating across sync/gpsimd right
    behind the hidden halves; phase-B matmuls wait on exactly the chunk
    they consume.
  - hidden supertile 1 (needed at mid-kernel) queued last.

Device layout (per core, T=1024 tokens):
  hiddenT [P, KC, T-tile] (host pre-transposed) so both big matmuls need
  no on-device transpose:
    phase A: latT[hd] = (latent_w[:, n*H+hd*128 : +128]).T @ hiddenT -> [128, T]
             tanh -> fp16
    phase B: logits = latT_tile.T @ output_w -> [128 tok, V] in PSUM
  softmax w/o max-subtract (logits ~ N(0, 0.63^2), exp is safe), denom via
  the ACT accum_out side-output; per-head combine is one fused DVE
  scalar_tensor_tensor: acc = (E * w_n) + acc.  The last head's combine +
  output DMA run in 512-column chunks to shorten the end-of-kernel drain.
"""

import os
import numpy as np
import ml_dtypes

B, S, H, NH, V = 4, 2048, 1024, 8, 2048
N_CORES = 8
T = (B * S) // N_CORES          # tokens per core
P = 128
KH = H // P                     # 8 contraction chunks
ST = 512                        # phase-A moving (token) tile
N_ST = T // ST
TT_PER_ST = ST // P
N_TT = T // P
VC = 512                        # logits free-dim chunk (one PSUM bank)
NVC = V // VC
NLQ = 4                         # latent_w quarter tiles per head
LQW = H // NLQ                  # 256 columns per quarter
EPS = 1e-8

_CACHE = {}


def _build(with_bias):
    import concourse.bass as bass
    import concourse.mybir as mybir
    import concourse.tile as tile
    from concourse import bacc
    from concourse.bass import ts

    f32 = mybir.dt.float32
    bf16 = mybir.dt.float16  # fp16: same PE rate as bf16, 8x finer mantissa

    KC = KH + (1 if with_bias else 0)   # contraction chunks incl. bias row
    HD = KH + (1 if with_bias else 0)   # logits contraction chunks
    KA = KC // 2                        # hidden kc-half split
    KB = KC - KA
    HPQ = KH // NLQ                     # hd slices per latent_w quarter

    nc = bacc.Bacc("TRN2", target_bir_lowering=False, debug=False)

    hTa_d = nc.dram_tensor("hiddenTa", [N_ST, P, KA, ST], bf16,
                           kind="ExternalInput")
    hTb_d = nc.dram_tensor("hiddenTb", [N_ST, P, KB, ST], bf16,
                           kind="ExternalInput")
    pw_d = nc.dram_tensor("prior_w", [P, KC, NH], bf16, kind="ExternalInput")
    lw_d = nc.dram_tensor("latent_w", [NH, NLQ, P, KC, LQW], bf16,
                          kind="ExternalInput")
    ow_d = nc.dram_tensor("output_w", [HD, P, V], bf16, kind="ExternalInput")
    out_d = nc.dram_tensor("out", [T, V], bf16, kind="ExternalOutput")

    with tile.TileContext(nc) as tc:
        with (
            tc.tile_pool(name="const", bufs=1) as const,
            tc.tile_pool(name="hid", bufs=N_ST) as hpool,
            tc.tile_pool(name="oww", bufs=1) as owpool,
            tc.tile_pool(name="pww", bufs=1) as pwpool,
            tc.tile_pool(name="lww", bufs=2) as lwpool,
            tc.tile_pool(name="lat", bufs=2 * KH) as latpool,
            tc.tile_pool(name="ee", bufs=3) as epool,
            tc.tile_pool(name="acc", bufs=TT_PER_ST + 1) as accpool,
            tc.tile_pool(name="small", bufs=4 * N_TT) as spool,
            tc.tile_pool(name="ps_lat", bufs=4, space="PSUM") as ps_lat,
            tc.tile_pool(name="ps_log", bufs=2, space="PSUM") as ps_log,
            tc.tile_pool(name="ps_pri", bufs=2, space="PSUM") as ps_pri,
        ):
            # ---- resident loads: priority order = first-use order --------
            pw = pwpool.tile([P, KC, NH], bf16, tag="pw")
            nc.sync.dma_start(pw[:], pw_d[:])

            hTs = []
            for sti in range(N_ST):
                t = hpool.tile([P, KC, ST], bf16, tag="hT")
                hTs.append(t)
            nc.sync.dma_start(hTs[0][:, 0:KA, :], hTa_d[0])
            nc.gpsimd.dma_start(hTs[0][:, KA:KC, :], hTb_d[0])

            def load_lw(n, engine=None):
                tiles = []
                for qi in range(NLQ):
                    t = lwpool.tile([P, KC, LQW], bf16, tag=f"lw{qi}")
                    (engine or nc.sync).dma_start(t[:], lw_d[n, qi])
                    tiles.append(t)
                return tiles

            lw_next = load_lw(0, engine=nc.scalar)

            # output_w per-hd chunks, alternating sync/gpsimd right behind
            # the hidden halves; consumed in hd order by phase B.
            ow_t = []
            for hd in range(HD):
                t = owpool.tile([P, V], bf16, tag=f"ow{hd}")
                (nc.sync if hd % 2 == 0 else nc.gpsimd).dma_start(
                    t[:], ow_d[hd])
                ow_t.append(t)

            # supertile-1 hidden: needed at mid-kernel, queued last.
            nc.gpsimd.dma_start(hTs[1][:, 0:KA, :], hTa_d[1])
            nc.gpsimd.dma_start(hTs[1][:, KA:KC, :], hTb_d[1])

            if with_bias:
                ones_t = const.tile([P, P], bf16)
                nc.vector.memset(ones_t[:], 0.0)
                nc.vector.memset(ones_t[0:1, :], 1.0)

            # ---- prior: wgt[tt] = sigmoid(h@pw) / (sum + EPS) -------------
            # emitted per supertile so the PE never waits on hT[st1]
            wgt = [None] * N_TT

            def prior_for(sti):
              for tt in range(sti * TT_PER_ST, (sti + 1) * TT_PER_ST):
                pr_ps = ps_pri.tile([P, NH], f32, tag="pri")
                for kc in range(KC):
                    nc.tensor.matmul(
                        pr_ps[:],
                        hTs[tt // TT_PER_ST][:, kc, ts(tt % TT_PER_ST, P)],
                        pw[:, kc, :],
                        start=(kc == 0),
                        stop=(kc == KC - 1),
                    )
                sig = spool.tile([P, NH], f32, tag="sig")
                ssum = spool.tile([P, 1], f32, tag="ssum")
                nc.scalar.activation(
                    sig[:], pr_ps[:], mybir.ActivationFunctionType.Sigmoid,
                    accum_out=ssum[:],
                )
                nc.vector.tensor_scalar_add(ssum[:], ssum[:], float(EPS))
                inv = spool.tile([P, 1], f32, tag="inv")
                nc.vector.reciprocal(inv[:], ssum[:])
                w = spool.tile([P, NH], f32, tag="wgt")
                nc.vector.tensor_scalar_mul(w[:], sig[:], inv[:])
                wgt[tt] = w

            # ---- main: per 512-token supertile, per head ------------------
            for st in range(N_ST):
                prior_for(st)
                acc = {}
                for n in range(NH):
                    # phase A: latT[hd] [128, ST] fp16 = tanh(lw_n.T @ hT_st)
                    lw_n = lw_next
                    if not (st == N_ST - 1 and n == NH - 1):
                        lw_next = load_lw((n + 1) % NH)
                    latT = []
                    for hd in range(KH):
                        lw_q = lw_n[hd // HPQ]
                        hcol = hd % HPQ
                        lat_ps = ps_lat.tile([P, ST], f32, tag="lat")
                        for kc in range(KC):
                            nc.tensor.matmul(
                                lat_ps[:],
                                lw_q[:, kc, ts(hcol, P)],
                                hTs[st][:, kc, :],
                                start=(kc == 0),
                                stop=(kc == KC - 1),
                            )
                        lt = latpool.tile([P, ST], bf16, tag="latT")
                        nc.scalar.activation(
                            lt[:], lat_ps[:], mybir.ActivationFunctionType.Tanh
                        )
                        latT.append(lt)

                    # phase B: per V-quarter [P,512] PSUM (1 bank,
                    # double-buffered): exp(q) overlaps the matmuls of q+1.
                    # E collects the full row; denom = reduce over the four
                    # per-quarter accum_outs.
                    for tti in range(TT_PER_ST):
                        tt = st * TT_PER_ST + tti
                        E = epool.tile([P, V], bf16, tag="E")
                        ds4 = spool.tile([P, NVC], f32, tag="ds4")
                        for q in range(NVC):
                            lg_ps = ps_log.tile([P, VC], f32, tag="log")
                            for hd in range(HD):
                                lhsT = (
                                    latT[hd][:, ts(tti, P)]
                                    if hd < KH
                                    else ones_t[:]
                                )
                                nc.tensor.matmul(
                                    lg_ps[:],
                                    lhsT,
                                    ow_t[hd][:, ts(q, VC)],
                                    start=(hd == 0),
                                    stop=(hd == HD - 1),
                                )
                            nc.scalar.activation(
                                E[:, ts(q, VC)], lg_ps[:],
                                mybir.ActivationFunctionType.Exp,
                                accum_out=ds4[:, q:q + 1],
                            )
                        dsm = spool.tile([P, 1], f32, tag="dsm")
                        nc.vector.tensor_reduce(
                            dsm[:], ds4[:], axis=mybir.AxisListType.X,
                            op=mybir.AluOpType.add,
                        )
                        invd = spool.tile([P, 1], f32, tag="invd")
                        nc.vector.reciprocal(invd[:], dsm[:])
                        wn = spool.tile([P, 1], f32, tag="wn")
                        nc.vector.tensor_tensor(
                            wn[:], wgt[tt][:, n:n + 1], invd[:],
                            op=mybir.AluOpType.mult,
                        )
                        if n == 0:
                            a = accpool.tile([P, V], bf16, tag="acc")
                            acc[tti] = a
                        else:
                            a = acc[tti]
                        # last head: 512-wide chunks so each chunk's output
                        # DMA starts as soon as its combine is done
                        # (shorter end-of-kernel drain); other heads: 1024.
                        nch = NVC if n == NH - 1 else 2
                        cw = V // nch
                        for ci in range(nch):
                            tgt = a[:, ts(ci, cw)]
                            Eh = E[:, ts(ci, cw)]
                            if n == 0:
                                nc.vector.tensor_scalar_mul(tgt, Eh, wn[:])
                            else:
                                nc.vector.scalar_tensor_tensor(
                                    tgt, Eh, wn[:], tgt,
                                    op0=mybir.AluOpType.mult,
                                    op1=mybir.AluOpType.add,
                                )
                            if n == NH - 1:
                                nc.sync.dma_start(
                                    out_d[ts(tt, P), ts(ci, cw)], tgt
                                )

    nc.compile()
    return nc


def _prep_inputs(hidden, prior_w, prior_b, latent_w, latent_b, output_w,
                 output_b, with_bias):
    """Rearrange inputs into the device (partition-major) layouts:
      hiddenTa/b [N_ST, P, KA/KB, ST] per core (kc-halves, contiguous)
      prior_w    [P, KC, NH]
      latent_w   [NH, NLQ, P, KC, LQW]  (H-quarters, contiguous)
      output_w   [HD, P, V]             (per-hd chunks, contiguous)
    """
    bf16 = np.float16
    KC = KH + (1 if with_bias else 0)
    HD = KH + (1 if with_bias else 0)
    KA = KC // 2
    BS = B * S

    h = hidden.reshape(BS, H).astype(bf16)
    pw = prior_w.astype(bf16)
    lw = latent_w.astype(bf16)
    ow = output_w.astype(bf16)
    if with_bias:
        hx = np.zeros((BS, P), bf16)
        hx[:, 0] = 1.0
        h = np.concatenate([h, hx], axis=1)                   # [BS, KC*P]
        pw = np.concatenate(
            [pw, prior_b.astype(bf16)[None, :], np.zeros((P - 1, NH), bf16)],
            axis=0)
        lw = np.concatenate(
            [lw, latent_b.astype(bf16)[None, :],
             np.zeros((P - 1, NH * H), bf16)], axis=0)
        ow = np.concatenate(
            [ow, output_b.astype(bf16)[None, :], np.zeros((P - 1, V), bf16)],
            axis=0)

    hT = h.reshape(BS, KC, P).transpose(2, 1, 0)              # [P, KC, BS]
    pw_dev = np.ascontiguousarray(pw.reshape(KC, P, NH).transpose(1, 0, 2))
    lw_dev = np.ascontiguousarray(
        lw.reshape(KC, P, NH, NLQ, LQW).transpose(2, 3, 1, 0, 4))
    ow_dev = np.ascontiguousarray(ow.reshape(HD, P, V))
    return hT, pw_dev, lw_dev, ow_dev, KA


def kernel(hidden, prior_w, prior_b, latent_w, latent_b, output_w, output_b,
           _profile=False):
    from concourse.bass_utils import run_bass_kernel_spmd

    # coerce to host numpy (the caller may hand us jax arrays)
    hidden = np.asarray(hidden, dtype=np.float32)
    prior_w = np.asarray(prior_w, dtype=np.float32)
    prior_b = np.asarray(prior_b, dtype=np.float32)
    latent_w = np.asarray(latent_w, dtype=np.float32)
    latent_b = np.asarray(latent_b, dtype=np.float32)
    output_w = np.asarray(output_w, dtype=np.float32)
    output_b = np.asarray(output_b, dtype=np.float32)

    with_bias = bool(
        np.any(prior_b) or np.any(latent_b) or np.any(output_b)
    )
    key = with_bias
    if key not in _CACHE:
        _CACHE[key] = _build(with_bias)
    nc = _CACHE[key]

    hT, pw, lw, ow, KA = _prep_inputs(
        hidden, prior_w, prior_b, latent_w, latent_b, output_w, output_b,
        with_bias)

    in_maps = []
    for c in range(N_CORES):
        hc = hT[:, :, c * T:(c + 1) * T]                      # [P, KC, T]
        hc = np.ascontiguousarray(
            hc.reshape(P, hc.shape[1], N_ST, ST).transpose(2, 0, 1, 3))
        in_maps.append({
            "hiddenTa": np.ascontiguousarray(hc[:, :, :KA, :]),
            "hiddenTb": np.ascontiguousarray(hc[:, :, KA:, :]),
            "prior_w": pw,
            "latent_w": lw,
            "output_w": ow,
        })

    res = run_bass_kernel_spmd(
        nc, in_maps, list(range(N_CORES)), trace=_profile
    )
    out = np.concatenate([res.results[c]["out"] for c in range(N_CORES)],
                         axis=0).astype(np.float32)
    if _profile:
        kernel.last_result = res
    return out.reshape(B, S, V)
